# revision 25
# baseline (speedup 1.0000x reference)
"""MoE (top-1 routing, E=8 experts) Trainium2 kernel.

Strategy (expert-parallel across 8 NeuronCores):
  - Routing (softmax/argmax/capacity) on host, replicating the reference
    op-for-op so expert assignment matches bit-exactly.
  - Each expert's capacity slots are SORTED by gate probability
    (descending, overflow/dummy slots last).  Low-p tokens contribute
    quadratically little to ||y||, so their FFN can run in fp8 (DoubleRow,
    ~1.8x PE rate) while staying inside the 2e-2 rel-err budget:
      * last N8G2=5 token-groups (640 tokens): GEMM2 in fp8
      * last N8G1=4 token-groups (512 tokens): GEMM1 also in fp8
  - All GEMM2 products carry a uniform 16*1024 scale (hT holds 16*h in
    fp16/fp8, W2 tiles hold 1024*W2), so fp16 and fp8 PSUM chains are
    evacuated identically with a single 1/16384 multiply.
  - Per-core kernel: Y_e = relu(Xe @ W1_e) @ W2_e with weights streamed
    through a 3-deep ring; per-F-block partials are streamed out as fp16
    and summed on host (as in the tuned fp16 baseline).
"""

import os
import sys

for _p in ("/opt/trn_rl_repo",):
    if os.path.isdir(_p) and _p not in sys.path:
        sys.path.insert(0, _p)

import numpy as np

B, S, D, F, E = 8, 2048, 1024, 4096, 8
T = B * S
CAP = T // E  # 2048, capacity_factor 1.0

F_BLK = 512          # F columns per outer block
N_FBLK = F // F_BLK  # 8
N_DC = D // 128      # 8 contraction chunks for GEMM1
N_FC = F_BLK // 128  # 4 f chunks per block
N_TG = CAP // 128    # 16 token groups

N8G2 = 5             # trailing token-groups with fp8 GEMM2
N8G1 = 4             # trailing token-groups with fp8 GEMM1 (<= N8G2, even)
T8G2 = N8G2 * 128    # 640 tokens
T8G1 = N8G1 * 128    # 512 tokens
T16 = CAP - T8G1     # 1536 tokens through fp16 GEMM1
TB = CAP - T8G2      # 1408: boundary between fp16/fp8 GEMM2 tokens
N_XC = T16 // 256    # 6 token chunks of the fp16 x stream


def _build_nc():
    import concourse.bacc as bacc
    import concourse.mybir as mybir
    from concourse.bass import ds
    from concourse.tile import TileContext

    f32 = mybir.dt.float32
    f16 = mybir.dt.float16
    f8 = mybir.dt.float8e4

    nc = bacc.Bacc("TRN2", target_bir_lowering=False, debug=False, num_devices=E)

    # Host pre-tiles inputs so DMA lines are contiguous per partition.
    xeT = nc.dram_tensor("xeT", [N_XC, 128, N_DC, 256], f16, kind="ExternalInput")
    xeT8 = nc.dram_tensor("xeT8", [128, N_DC // 2, 2, T8G1], f8, kind="ExternalInput")
    w1 = nc.dram_tensor("w1", [N_FBLK, 128, N_DC, F_BLK], f16, kind="ExternalInput")
    w18 = nc.dram_tensor("w18", [N_FBLK, 128, N_DC // 2, 2, F_BLK], f8, kind="ExternalInput")
    # fp16 W2 tiles carry a 1024x scale; fp8 carries the same.
    w2 = nc.dram_tensor("w2", [N_FBLK, 128, N_FC, D], f16, kind="ExternalInput")
    w28 = nc.dram_tensor("w28", [N_FBLK, 128, N_FC // 2, 2, D], f8, kind="ExternalInput")
    # Per-F-block GEMM2 partials; host sums over axis 0.
    y = nc.dram_tensor("y", [N_FBLK, CAP, D], f16, kind="ExternalOutput")

    x_r = xeT.ap().rearrange("c p dc j -> p c dc j")
    w1_r = w1.ap().rearrange("fo p dc j -> p fo dc j")
    w18_r = w18.ap().rearrange("fo p q s j -> p fo q s j")
    w2_r = w2.ap().rearrange("fo p fc d -> p fo fc d")
    w28_r = w28.ap().rearrange("fo p q s d -> p fo q s d")
    y_r = y.ap().rearrange("fo (tg p) d -> p fo tg d", p=128)

    with TileContext(nc) as tc:
        with (
            tc.tile_pool(name="sbuf", bufs=1) as sbuf,
            tc.tile_pool(name="wpool", bufs=3) as wpool,
            tc.tile_pool(name="spool", bufs=3) as spool,
            tc.tile_pool(name="psh", bufs=3, space="PSUM") as psh,
            tc.tile_pool(name="psy", bufs=5, space="PSUM") as psy,
        ):
            # PE warmup: dependency-light fp32 matmuls ramp the HAM clock
            # while the first DMAs land.
            warm_sb = sbuf.tile([128, 384], f32, tag="warm")
            nc.vector.memset(warm_sb, 0)
            for _ in range(13):
                pwarm = psy.tile([128, 512], f32, tag="py")
                nc.tensor.matmul(
                    pwarm[:, :256], warm_sb[:, :128], warm_sb[:, ds(128, 256)],
                    start=True, stop=True,
                )

            xeT_sb = sbuf.tile([128, N_DC, T16], f16, tag="x")
            xeT8_sb = sbuf.tile([128, N_DC // 2, 2, T8G1], f8, tag="x8")

            # hT holds 16*relu(.) for fp16-GEMM2 tokens; hT8 the same in fp8
            # for the trailing T8G2 tokens.
            hT = sbuf.tile([128, N_FC, TB], f16, tag="h")
            hT8 = sbuf.tile([128, N_FC // 2, 2, T8G2], f8, tag="h8")

            w1_tiles = [None] * N_FBLK
            w18_tiles = [None] * N_FBLK
            w2_tiles = [None] * N_FBLK
            w28_tiles = [None] * N_FBLK

            def fetch_w1(fo):
                t = wpool.tile([128, N_DC, F_BLK], f16, tag="w1")
                nc.sync.dma_start(out=t, in_=w1_r[:, fo, :, :])
                w1_tiles[fo] = t

            def fetch_w18(fo, eng=None):
                t = wpool.tile([128, N_DC // 2, 2, F_BLK], f8, tag="w18")
                (eng or nc.sync).dma_start(out=t, in_=w18_r[:, fo, :, :, :])
                w18_tiles[fo] = t

            def fetch_w2(fo, eng=None):
                t = wpool.tile([128, N_FC, D], f16, tag="w2")
                (eng or nc.sync).dma_start(out=t, in_=w2_r[:, fo, :, :])
                w2_tiles[fo] = t

            def fetch_w28(fo, eng=None):
                t = wpool.tile([128, N_FC // 2, 2, D], f8, tag="w28")
                (eng or nc.sync).dma_start(out=t, in_=w28_r[:, fo, :, :, :])
                w28_tiles[fo] = t

            # Startup streaming: critical fo0 operands go first as few, large
            # DMAs (queue-issue costs ~0.6us each).  gpsimd (SWDGE) is
            # fastest during the startup window; scalar HWDGE is idle early
            # and takes a share of the x stream.
            # fo0's W1 is fetched as two halves on parallel queues so the
            # first GEMM1 chain unblocks ~2x sooner after the startup
            # barrier releases the DMA queues.
            w1t0 = wpool.tile([128, N_DC, F_BLK], f16, tag="w1")
            nc.sync.dma_start(out=w1t0[:, ds(0, 4), :], in_=w1_r[:, 0, ds(0, 4), :])
            nc.scalar.dma_start(out=w1t0[:, ds(4, 4), :], in_=w1_r[:, 0, ds(4, 4), :])
            w1_tiles[0] = w1t0
            nc.gpsimd.dma_start(out=xeT_sb[:, :, ds(0, 256)], in_=x_r[:, 0, :, :])
            nc.scalar.dma_start(
                out=xeT_sb[:, :, ds(1 * 256, 256)], in_=x_r[:, 1, :, :])
            nc.sync.dma_start(
                out=xeT_sb[:, :, ds(2 * 256, 256)], in_=x_r[:, 2, :, :])
            nc.gpsimd.dma_start(out=xeT8_sb, in_=xeT8.ap())
            fetch_w18(0, eng=nc.gpsimd)
            nc.scalar.dma_start(
                out=xeT_sb[:, :, ds(3 * 256, 256)], in_=x_r[:, 3, :, :])
            nc.gpsimd.dma_start(
                out=xeT_sb[:, :, ds(4 * 256, 256)], in_=x_r[:, 4, :, :])
            fetch_w28(0, eng=nc.gpsimd)
            fetch_w1(1)
            nc.sync.dma_start(
                out=xeT_sb[:, :, ds(5 * 256, 256)], in_=x_r[:, 5, :, :])
            fetch_w2(0, eng=nc.scalar)
            fetch_w2(1)
            fetch_w18(1)
            fetch_w28(1)

            for fo in range(N_FBLK):
                # Prefetch weights two blocks ahead (ring depth 3).
                if fo + 2 < N_FBLK:
                    fetch_w1(fo + 2)
                    fetch_w2(fo + 2)
                    fetch_w18(fo + 2)
                    fetch_w28(fo + 2)
                w1t = w1_tiles[fo]
                w18t = w18_tiles[fo]
                nq = N_DC // 2

                def g1_col(tcix, tok_w):
                    # GEMM1 fp16 column of tok_w tokens
                    t0 = tcix * tok_w
                    for fc in range(N_FC):
                        ph = psh.tile([128, 512], f32, tag="ph")
                        for dc in range(N_DC):
                            nc.tensor.matmul(
                                ph[:, :tok_w],
                                w1t[:, dc, ds(fc * 128, 128)],
                                xeT_sb[:, dc, ds(t0, tok_w)],
                                start=(dc == 0),
                                stop=(dc == N_DC - 1),
                            )
                        if t0 + tok_w <= TB:
                            nc.scalar.activation(
                                hT[:, fc, ds(t0, tok_w)], ph[:, :tok_w],
                                mybir.ActivationFunctionType.Relu, scale=16.0,
                            )
                        else:
                            # column straddles the fp16/fp8 GEMM2 boundary
                            wb = TB - t0
                            nc.scalar.activation(
                                hT[:, fc, ds(t0, wb)], ph[:, :wb],
                                mybir.ActivationFunctionType.Relu, scale=16.0,
                            )
                            nc.scalar.activation(
                                hT8[:, fc // 2, fc % 2, ds(0, tok_w - wb)],
                                ph[:, ds(wb, tok_w - wb)],
                                mybir.ActivationFunctionType.Relu, scale=16.0,
                            )

                def g1_fp8():
                    # GEMM1 fp8 tokens [T16, CAP): 2 cols of 256.  Operands
                    # carry 16x (x) and 64x (W1) scales -> psum is
                    # 1024*(xe@W1); evacuate with scale 16/1024 for 16*h.
                    # col1 accumulates q in reverse so consecutive chains
                    # share a stationary (DR LDWEIGHTS cannot hide behind
                    # an in-flight DR matmul).
                    for fc in range(N_FC):
                        ph8_0 = psh.tile([128, 512], f32, tag="ph")
                        ph8_1 = psh.tile([128, 512], f32, tag="ph")
                        ph8 = [ph8_0, ph8_1]
                        for col in range(2):
                            qseq = (range(nq) if col == 0
                                    else range(nq - 1, -1, -1))
                            for k, q in enumerate(qseq):
                                nc.tensor.matmul(
                                    ph8[col][:, :256],
                                    w18t[:, q, :, ds(fc * 128, 128)],
                                    xeT8_sb[:, q, :, ds(col * 256, 256)],
                                    start=(k == 0),
                                    stop=(k == nq - 1),
                                    perf_mode=mybir.MatmulPerfMode.DoubleRow,
                                )
                        for col in range(2):
                            nc.scalar.activation(
                                hT8[:, fc // 2, fc % 2,
                                    ds(T8G2 - T8G1 + col * 256, 256)],
                                ph8[col][:, :256],
                                mybir.ActivationFunctionType.Relu,
                                scale=1.0 / 64.0,
                            )

                # fo==0 walks 256-token columns to match streaming xeT
                # arrival, with the fp8 section (fed by the independent
                # gpsimd queue) interleaved as alternative PE work in case
                # the x stream lags; later blocks use 512-wide columns.
                if fo == 0:
                    for tcix in range(4):
                        g1_col(tcix, 256)
                    g1_fp8()
                    for tcix in range(4, N_XC):
                        g1_col(tcix, 256)
                else:
                    for tcix in range(T16 // 512):
                        g1_col(tcix, 512)
                    g1_fp8()

                # ---- GEMM2: per token-group; products carry 16384x ----
                # fp8 token-groups are emitted between fp16 pairs so their
                # DR LDWEIGHTS can hide behind fp16 matmuls; the dh1 chain
                # accumulates q in reverse to share stationaries.
                w2t = w2_tiles[fo]
                w28t = w28_tiles[fo]
                last = fo == N_FBLK - 1
                for tg in range(N_TG):
                    if tg % 2 == 0:
                        stage = spool.tile([128, 2, D], f16, tag="st")
                    py0 = psy.tile([128, 512], f32, tag="py")
                    py1 = psy.tile([128, 512], f32, tag="py")
                    if tg * 128 < TB:
                        for fc in range(N_FC):
                            lhs = hT[:, fc, ds(tg * 128, 128)]
                            nc.tensor.matmul(
                                py0, lhs, w2t[:, fc, ds(0, 512)],
                                start=(fc == 0), stop=(fc == N_FC - 1),
                            )
                            nc.tensor.matmul(
                                py1, lhs, w2t[:, fc, ds(512, 512)],
                                start=(fc == 0), stop=(fc == N_FC - 1),
                            )
                    else:
                        t8 = tg * 128 - TB
                        nqf = N_FC // 2
                        for dh, py in ((0, py0), (1, py1)):
                            qseq = (range(nqf) if dh == 0
                                    else range(nqf - 1, -1, -1))
                            for k, q in enumerate(qseq):
                                nc.tensor.matmul(
                                    py, hT8[:, q, :, ds(t8, 128)],
                                    w28t[:, q, :, ds(dh * 512, 512)],
                                    start=(k == 0), stop=(k == nqf - 1),
                                    perf_mode=mybir.MatmulPerfMode.DoubleRow,
                                )
                    nc.vector.tensor_scalar_mul(
                        stage[:, tg % 2, ds(0, 512)], py0, 1.0 / 16384.0)
                    nc.vector.tensor_scalar_mul(
                        stage[:, tg % 2, ds(512, 512)], py1, 1.0 / 16384.0)
                    # weight prefetches own the sync queue mid-kernel, so
                    # y goes out on scalar; the final block stores per-tg
                    # on both queues so the tail drains fast.
                    if last:
                        eng = nc.scalar if tg % 2 == 0 else nc.sync
                        eng.dma_start(
                            out=y_r[:, fo, ds(tg, 1), :],
                            in_=stage[:, ds(tg % 2, 1), :])
                    elif tg % 2 == 1:
                        nc.scalar.dma_start(
                            out=y_r[:, fo, ds(tg - 1, 2), :], in_=stage)

    nc.compile()
    return nc


_NC = None


def _get_nc():
    global _NC
    if _NC is None:
        _NC = _build_nc()
    return _NC


def _route(xf, Wr):
    """Replicates the reference routing (jax-on-CPU, op-for-op) so that
    expert assignment matches the fp32 reference bit-exactly."""
    try:
        import jax
        import jax.numpy as jnp

        cpu = jax.local_devices(backend="cpu")[0]
        with jax.default_device(cpu):
            xj = jnp.asarray(xf, dtype=jnp.float32)
            wj = jnp.asarray(Wr, dtype=jnp.float32)
            probs = jax.nn.softmax(xj @ wj, axis=-1)
            eidx_j = jnp.argmax(probs, axis=-1)
            p_tok_j = jnp.take_along_axis(probs, eidx_j[:, None], axis=1)[:, 0]
            eidx = np.asarray(eidx_j)
            p_tok = np.asarray(p_tok_j)
    except Exception:
        logits = xf.astype(np.float32) @ Wr.astype(np.float32)
        lmax = logits.max(axis=-1, keepdims=True)
        ex = np.exp(logits - lmax)
        probs = ex / ex.sum(axis=-1, keepdims=True)
        eidx = np.argmax(probs, axis=-1)
        p_tok = probs[np.arange(T), eidx]

    onehot = np.zeros((T, E), dtype=np.int64)
    onehot[np.arange(T), eidx] = 1
    rank = np.cumsum(onehot, axis=0) - onehot
    rank = rank[np.arange(T), eidx]
    keep = rank < CAP

    dispatch = np.zeros((E, CAP), dtype=np.int64)
    valid = np.zeros((E, CAP), dtype=bool)
    kept = np.nonzero(keep)[0]
    dispatch[eidx[kept], rank[kept]] = kept
    valid[eidx[kept], rank[kept]] = True
    return dispatch, valid, p_tok


def kernel(x, Wr, W1, W2):
    from concourse.bass_utils import run_bass_kernel_spmd
    import ml_dtypes

    F8 = ml_dtypes.float8_e4m3

    x = np.asarray(x, dtype=np.float32)
    Wr = np.asarray(Wr, dtype=np.float32)
    W1 = np.asarray(W1, dtype=np.float32)
    W2 = np.asarray(W2, dtype=np.float32)

    xf = x.reshape(T, D)
    dispatch, valid, p_tok = _route(xf, Wr)

    in_maps = []
    disp_s, valid_s = [], []
    for e in range(E):
        # sort capacity slots by gate probability (desc); dummies last
        pe = np.where(valid[e], p_tok[dispatch[e]], -1.0)
        order = np.argsort(-pe, kind="stable")
        de, ve = dispatch[e][order], valid[e][order]
        disp_s.append(de)
        valid_s.append(ve)
        scale = np.where(ve, p_tok[de], 0.0).astype(np.float32)
        xe = xf[de] * scale[:, None]  # [CAP, D]; relu(s*x@W1)@W2 = s*y

        # fp16 x stream: first T16 tokens, [c, p, dc, 256]
        xeT_t = (xe[:T16].T.astype(np.float16)
                 .reshape(N_DC, 128, N_XC, 256).transpose(2, 1, 0, 3))
        # fp8 x: last T8G1 tokens, 16x scale, [p, dcpair, slot, tok]
        xe8 = (16.0 * xe[T16:]).astype(F8)
        xeT8_t = xe8.T.reshape(N_DC // 2, 2, 128, T8G1).transpose(2, 0, 1, 3)

        w1_t = (W1[e].astype(np.float16)
                .reshape(N_DC, 128, N_FBLK, F_BLK).transpose(2, 1, 0, 3))
        w18_f = (64.0 * W1[e]).astype(F8)
        w18_t = (w18_f.reshape(N_DC // 2, 2, 128, N_FBLK, F_BLK)
                 .transpose(3, 2, 0, 1, 4))

        w2s = 1024.0 * W2[e]
        w2_t = (w2s.astype(np.float16)
                .reshape(N_FBLK, N_FC, 128, D).transpose(0, 2, 1, 3))
        w28_t = (w2s.astype(F8)
                 .reshape(N_FBLK, N_FC // 2, 2, 128, D).transpose(0, 3, 1, 2, 4))

        in_maps.append({
            "xeT": np.ascontiguousarray(xeT_t),
            "xeT8": np.ascontiguousarray(xeT8_t),
            "w1": np.ascontiguousarray(w1_t),
            "w18": np.ascontiguousarray(w18_t),
            "w2": np.ascontiguousarray(w2_t),
            "w28": np.ascontiguousarray(w28_t),
        })

    nc = _get_nc()
    res = run_bass_kernel_spmd(nc, in_maps, core_ids=list(range(E)))

    yf = np.zeros((T, D), dtype=np.float32)
    for e in range(E):
        ye = res.results[e]["y"].astype(np.float32).sum(axis=0)  # [CAP, D]
        m = valid_s[e]
        yf[disp_s[e][m]] = ye[m]
    return yf.reshape(B, S, D)


# revision 26
# speedup vs baseline: 1.0113x; 1.0113x over previous
"""MoE (top-1 routing, E=8 experts) Trainium2 kernel.

Strategy (expert-parallel across 8 NeuronCores):
  - Routing (softmax/argmax/capacity) on host, replicating the reference
    op-for-op so expert assignment matches bit-exactly.
  - Each expert's capacity slots are SORTED by gate probability
    (descending, overflow/dummy slots last).  Low-p tokens contribute
    quadratically little to ||y||, so their FFN can run in fp8 (DoubleRow,
    ~1.8x PE rate) while staying inside the 2e-2 rel-err budget:
      * last N8G2=5 token-groups (640 tokens): GEMM2 in fp8
      * last N8G1=4 token-groups (512 tokens): GEMM1 also in fp8
  - All GEMM2 products carry a uniform 16*1024 scale (hT holds 16*h in
    fp16/fp8, W2 tiles hold 1024*W2), so fp16 and fp8 PSUM chains are
    evacuated identically with a single 1/16384 multiply.
  - Per-core kernel: Y_e = relu(Xe @ W1_e) @ W2_e with weights streamed
    through a 3-deep ring; per-F-block partials are streamed out as fp16
    and summed on host (as in the tuned fp16 baseline).
"""

import os
import sys

for _p in ("/opt/trn_rl_repo",):
    if os.path.isdir(_p) and _p not in sys.path:
        sys.path.insert(0, _p)

import numpy as np

B, S, D, F, E = 8, 2048, 1024, 4096, 8
T = B * S
CAP = T // E  # 2048, capacity_factor 1.0

F_BLK = 512          # F columns per outer block
N_FBLK = F // F_BLK  # 8
N_DC = D // 128      # 8 contraction chunks for GEMM1
N_FC = F_BLK // 128  # 4 f chunks per block
N_TG = CAP // 128    # 16 token groups

N8G2 = 5             # trailing token-groups with fp8 GEMM2
N8G1 = 4             # trailing token-groups with fp8 GEMM1 (<= N8G2, even)
T8G2 = N8G2 * 128    # 640 tokens
T8G1 = N8G1 * 128    # 512 tokens
T16 = CAP - T8G1     # 1536 tokens through fp16 GEMM1
TB = CAP - T8G2      # 1408: boundary between fp16/fp8 GEMM2 tokens
N_XC = T16 // 256    # 6 token chunks of the fp16 x stream


def _build_nc():
    import concourse.bacc as bacc
    import concourse.mybir as mybir
    from concourse.bass import ds
    from concourse.tile import TileContext

    f32 = mybir.dt.float32
    f16 = mybir.dt.float16
    f8 = mybir.dt.float8e4

    nc = bacc.Bacc("TRN2", target_bir_lowering=False, debug=False, num_devices=E)

    # Host pre-tiles inputs so DMA lines are contiguous per partition.
    xeT = nc.dram_tensor("xeT", [N_XC, 128, N_DC, 256], f16, kind="ExternalInput")
    xeT8 = nc.dram_tensor("xeT8", [128, N_DC // 2, 2, T8G1], f8, kind="ExternalInput")
    w1 = nc.dram_tensor("w1", [N_FBLK, 128, N_DC, F_BLK], f16, kind="ExternalInput")
    w18 = nc.dram_tensor("w18", [N_FBLK, 128, N_DC // 2, 2, F_BLK], f8, kind="ExternalInput")
    # fp16 W2 tiles carry a 1024x scale; fp8 carries the same.
    w2 = nc.dram_tensor("w2", [N_FBLK, 128, N_FC, D], f16, kind="ExternalInput")
    w28 = nc.dram_tensor("w28", [N_FBLK, 128, N_FC // 2, 2, D], f8, kind="ExternalInput")
    # Per-F-block GEMM2 partials; host sums over axis 0.
    y = nc.dram_tensor("y", [N_FBLK, CAP, D], f16, kind="ExternalOutput")

    x_r = xeT.ap().rearrange("c p dc j -> p c dc j")
    w1_r = w1.ap().rearrange("fo p dc j -> p fo dc j")
    w18_r = w18.ap().rearrange("fo p q s j -> p fo q s j")
    w2_r = w2.ap().rearrange("fo p fc d -> p fo fc d")
    w28_r = w28.ap().rearrange("fo p q s d -> p fo q s d")
    y_r = y.ap().rearrange("fo (tg p) d -> p fo tg d", p=128)

    with TileContext(nc) as tc:
        with (
            tc.tile_pool(name="sbuf", bufs=1) as sbuf,
            tc.tile_pool(name="wpool", bufs=3) as wpool,
            tc.tile_pool(name="spool", bufs=3) as spool,
            tc.tile_pool(name="psh", bufs=3, space="PSUM") as psh,
            tc.tile_pool(name="psy", bufs=5, space="PSUM") as psy,
        ):
            # PE warmup: dependency-light fp32 matmuls ramp the HAM clock
            # while the first DMAs land.
            warm_sb = sbuf.tile([128, 384], f32, tag="warm")
            nc.vector.memset(warm_sb, 0)
            for _ in range(13):
                pwarm = psy.tile([128, 512], f32, tag="py")
                nc.tensor.matmul(
                    pwarm[:, :256], warm_sb[:, :128], warm_sb[:, ds(128, 256)],
                    start=True, stop=True,
                )

            xeT_sb = sbuf.tile([128, N_DC, T16], f16, tag="x")
            xeT8_sb = sbuf.tile([128, N_DC // 2, 2, T8G1], f8, tag="x8")

            # hT holds 16*relu(.) for fp16-GEMM2 tokens; hT8 the same in fp8
            # for the trailing T8G2 tokens.
            hT = sbuf.tile([128, N_FC, TB], f16, tag="h")
            hT8 = sbuf.tile([128, N_FC // 2, 2, T8G2], f8, tag="h8")

            w1_tiles = [None] * N_FBLK
            w18_tiles = [None] * N_FBLK
            w2_tiles = [None] * N_FBLK
            w28_tiles = [None] * N_FBLK

            def fetch_w1(fo):
                t = wpool.tile([128, N_DC, F_BLK], f16, tag="w1")
                nc.sync.dma_start(out=t, in_=w1_r[:, fo, :, :])
                w1_tiles[fo] = t

            def fetch_w18(fo, eng=None):
                t = wpool.tile([128, N_DC // 2, 2, F_BLK], f8, tag="w18")
                (eng or nc.sync).dma_start(out=t, in_=w18_r[:, fo, :, :, :])
                w18_tiles[fo] = t

            def fetch_w2(fo, eng=None):
                t = wpool.tile([128, N_FC, D], f16, tag="w2")
                (eng or nc.sync).dma_start(out=t, in_=w2_r[:, fo, :, :])
                w2_tiles[fo] = t

            def fetch_w28(fo, eng=None):
                t = wpool.tile([128, N_FC // 2, 2, D], f8, tag="w28")
                (eng or nc.sync).dma_start(out=t, in_=w28_r[:, fo, :, :, :])
                w28_tiles[fo] = t

            # Startup streaming: critical fo0 operands go first as few, large
            # DMAs (queue-issue costs ~0.6us each).  gpsimd (SWDGE) is
            # fastest during the startup window; scalar HWDGE is idle early
            # and takes a share of the x stream.
            # fo0's W1 is fetched as two halves on parallel queues so the
            # first GEMM1 chain unblocks ~2x sooner after the startup
            # barrier releases the DMA queues.
            w1t0 = wpool.tile([128, N_DC, F_BLK], f16, tag="w1")
            nc.sync.dma_start(out=w1t0[:, ds(0, 4), :], in_=w1_r[:, 0, ds(0, 4), :])
            nc.scalar.dma_start(out=w1t0[:, ds(4, 4), :], in_=w1_r[:, 0, ds(4, 4), :])
            w1_tiles[0] = w1t0
            nc.gpsimd.dma_start(out=xeT_sb[:, :, ds(0, 256)], in_=x_r[:, 0, :, :])
            nc.scalar.dma_start(
                out=xeT_sb[:, :, ds(1 * 256, 256)], in_=x_r[:, 1, :, :])
            nc.sync.dma_start(
                out=xeT_sb[:, :, ds(2 * 256, 256)], in_=x_r[:, 2, :, :])
            nc.gpsimd.dma_start(out=xeT8_sb, in_=xeT8.ap())
            fetch_w18(0, eng=nc.gpsimd)
            nc.scalar.dma_start(
                out=xeT_sb[:, :, ds(3 * 256, 256)], in_=x_r[:, 3, :, :])
            nc.gpsimd.dma_start(
                out=xeT_sb[:, :, ds(4 * 256, 256)], in_=x_r[:, 4, :, :])
            fetch_w28(0, eng=nc.gpsimd)
            fetch_w1(1)
            nc.sync.dma_start(
                out=xeT_sb[:, :, ds(5 * 256, 256)], in_=x_r[:, 5, :, :])
            fetch_w2(0, eng=nc.scalar)
            fetch_w2(1)
            fetch_w18(1)
            fetch_w28(1)

            for fo in range(N_FBLK):
                # Prefetch weights two blocks ahead (ring depth 3).
                if fo + 2 < N_FBLK:
                    fetch_w1(fo + 2)
                    fetch_w2(fo + 2)
                    fetch_w18(fo + 2)
                    fetch_w28(fo + 2)
                w1t = w1_tiles[fo]
                w18t = w18_tiles[fo]
                nq = N_DC // 2

                def g1_col(tcix, tok_w):
                    # GEMM1 fp16 column of tok_w tokens
                    t0 = tcix * tok_w
                    for fc in range(N_FC):
                        ph = psh.tile([128, 512], f32, tag="ph")
                        for dc in range(N_DC):
                            nc.tensor.matmul(
                                ph[:, :tok_w],
                                w1t[:, dc, ds(fc * 128, 128)],
                                xeT_sb[:, dc, ds(t0, tok_w)],
                                start=(dc == 0),
                                stop=(dc == N_DC - 1),
                            )
                        if t0 + tok_w <= TB:
                            nc.scalar.activation(
                                hT[:, fc, ds(t0, tok_w)], ph[:, :tok_w],
                                mybir.ActivationFunctionType.Relu, scale=16.0,
                            )
                        else:
                            # column straddles the fp16/fp8 GEMM2 boundary
                            wb = TB - t0
                            nc.scalar.activation(
                                hT[:, fc, ds(t0, wb)], ph[:, :wb],
                                mybir.ActivationFunctionType.Relu, scale=16.0,
                            )
                            nc.scalar.activation(
                                hT8[:, fc // 2, fc % 2, ds(0, tok_w - wb)],
                                ph[:, ds(wb, tok_w - wb)],
                                mybir.ActivationFunctionType.Relu, scale=16.0,
                            )

                def g1_fp8():
                    # GEMM1 fp8 tokens [T16, CAP): 2 cols of 256.  Operands
                    # carry 16x (x) and 64x (W1) scales -> psum is
                    # 1024*(xe@W1); evacuate with scale 16/1024 for 16*h.
                    # col1 accumulates q in reverse so consecutive chains
                    # share a stationary (DR LDWEIGHTS cannot hide behind
                    # an in-flight DR matmul).
                    for fc in range(N_FC):
                        ph8_0 = psh.tile([128, 512], f32, tag="ph")
                        ph8_1 = psh.tile([128, 512], f32, tag="ph")
                        ph8 = [ph8_0, ph8_1]
                        for col in range(2):
                            qseq = (range(nq) if col == 0
                                    else range(nq - 1, -1, -1))
                            for k, q in enumerate(qseq):
                                nc.tensor.matmul(
                                    ph8[col][:, :256],
                                    w18t[:, q, :, ds(fc * 128, 128)],
                                    xeT8_sb[:, q, :, ds(col * 256, 256)],
                                    start=(k == 0),
                                    stop=(k == nq - 1),
                                    perf_mode=mybir.MatmulPerfMode.DoubleRow,
                                )
                        for col in range(2):
                            nc.scalar.activation(
                                hT8[:, fc // 2, fc % 2,
                                    ds(T8G2 - T8G1 + col * 256, 256)],
                                ph8[col][:, :256],
                                mybir.ActivationFunctionType.Relu,
                                scale=1.0 / 64.0,
                            )

                # fo==0 walks 256-token columns to match streaming xeT
                # arrival, with the fp8 section (fed by the independent
                # gpsimd queue) interleaved as alternative PE work in case
                # the x stream lags; later blocks use 512-wide columns.
                if fo == 0:
                    for tcix in range(N_XC):
                        g1_col(tcix, 256)
                    g1_fp8()
                else:
                    for tcix in range(T16 // 512):
                        g1_col(tcix, 512)
                    g1_fp8()

                # ---- GEMM2: per token-group; products carry 16384x ----
                # fp8 token-groups are emitted between fp16 pairs so their
                # DR LDWEIGHTS can hide behind fp16 matmuls; the dh1 chain
                # accumulates q in reverse to share stationaries.
                w2t = w2_tiles[fo]
                w28t = w28_tiles[fo]
                last = fo == N_FBLK - 1
                for tg in range(N_TG):
                    if tg % 2 == 0:
                        stage = spool.tile([128, 2, D], f16, tag="st")
                    py0 = psy.tile([128, 512], f32, tag="py")
                    py1 = psy.tile([128, 512], f32, tag="py")
                    if tg * 128 < TB:
                        for fc in range(N_FC):
                            lhs = hT[:, fc, ds(tg * 128, 128)]
                            nc.tensor.matmul(
                                py0, lhs, w2t[:, fc, ds(0, 512)],
                                start=(fc == 0), stop=(fc == N_FC - 1),
                            )
                            nc.tensor.matmul(
                                py1, lhs, w2t[:, fc, ds(512, 512)],
                                start=(fc == 0), stop=(fc == N_FC - 1),
                            )
                    else:
                        t8 = tg * 128 - TB
                        nqf = N_FC // 2
                        for dh, py in ((0, py0), (1, py1)):
                            qseq = (range(nqf) if dh == 0
                                    else range(nqf - 1, -1, -1))
                            for k, q in enumerate(qseq):
                                nc.tensor.matmul(
                                    py, hT8[:, q, :, ds(t8, 128)],
                                    w28t[:, q, :, ds(dh * 512, 512)],
                                    start=(k == 0), stop=(k == nqf - 1),
                                    perf_mode=mybir.MatmulPerfMode.DoubleRow,
                                )
                    nc.vector.tensor_scalar_mul(
                        stage[:, tg % 2, ds(0, 512)], py0, 1.0 / 16384.0)
                    nc.vector.tensor_scalar_mul(
                        stage[:, tg % 2, ds(512, 512)], py1, 1.0 / 16384.0)
                    # weight prefetches own the sync queue mid-kernel, so
                    # y goes out on scalar; the final block stores per-tg
                    # on both queues so the tail drains fast.
                    if last:
                        eng = nc.scalar if tg % 2 == 0 else nc.sync
                        eng.dma_start(
                            out=y_r[:, fo, ds(tg, 1), :],
                            in_=stage[:, ds(tg % 2, 1), :])
                    elif tg % 2 == 1:
                        nc.scalar.dma_start(
                            out=y_r[:, fo, ds(tg - 1, 2), :], in_=stage)

    nc.compile()
    return nc


_NC = None


def _get_nc():
    global _NC
    if _NC is None:
        _NC = _build_nc()
    return _NC


def _route(xf, Wr):
    """Replicates the reference routing (jax-on-CPU, op-for-op) so that
    expert assignment matches the fp32 reference bit-exactly."""
    try:
        import jax
        import jax.numpy as jnp

        cpu = jax.local_devices(backend="cpu")[0]
        with jax.default_device(cpu):
            xj = jnp.asarray(xf, dtype=jnp.float32)
            wj = jnp.asarray(Wr, dtype=jnp.float32)
            probs = jax.nn.softmax(xj @ wj, axis=-1)
            eidx_j = jnp.argmax(probs, axis=-1)
            p_tok_j = jnp.take_along_axis(probs, eidx_j[:, None], axis=1)[:, 0]
            eidx = np.asarray(eidx_j)
            p_tok = np.asarray(p_tok_j)
    except Exception:
        logits = xf.astype(np.float32) @ Wr.astype(np.float32)
        lmax = logits.max(axis=-1, keepdims=True)
        ex = np.exp(logits - lmax)
        probs = ex / ex.sum(axis=-1, keepdims=True)
        eidx = np.argmax(probs, axis=-1)
        p_tok = probs[np.arange(T), eidx]

    onehot = np.zeros((T, E), dtype=np.int64)
    onehot[np.arange(T), eidx] = 1
    rank = np.cumsum(onehot, axis=0) - onehot
    rank = rank[np.arange(T), eidx]
    keep = rank < CAP

    dispatch = np.zeros((E, CAP), dtype=np.int64)
    valid = np.zeros((E, CAP), dtype=bool)
    kept = np.nonzero(keep)[0]
    dispatch[eidx[kept], rank[kept]] = kept
    valid[eidx[kept], rank[kept]] = True
    return dispatch, valid, p_tok


def kernel(x, Wr, W1, W2):
    from concourse.bass_utils import run_bass_kernel_spmd
    import ml_dtypes

    F8 = ml_dtypes.float8_e4m3

    x = np.asarray(x, dtype=np.float32)
    Wr = np.asarray(Wr, dtype=np.float32)
    W1 = np.asarray(W1, dtype=np.float32)
    W2 = np.asarray(W2, dtype=np.float32)

    xf = x.reshape(T, D)
    dispatch, valid, p_tok = _route(xf, Wr)

    in_maps = []
    disp_s, valid_s = [], []
    for e in range(E):
        # sort capacity slots by gate probability (desc); dummies last
        pe = np.where(valid[e], p_tok[dispatch[e]], -1.0)
        order = np.argsort(-pe, kind="stable")
        de, ve = dispatch[e][order], valid[e][order]
        disp_s.append(de)
        valid_s.append(ve)
        scale = np.where(ve, p_tok[de], 0.0).astype(np.float32)
        xe = xf[de] * scale[:, None]  # [CAP, D]; relu(s*x@W1)@W2 = s*y

        # fp16 x stream: first T16 tokens, [c, p, dc, 256]
        xeT_t = (xe[:T16].T.astype(np.float16)
                 .reshape(N_DC, 128, N_XC, 256).transpose(2, 1, 0, 3))
        # fp8 x: last T8G1 tokens, 16x scale, [p, dcpair, slot, tok]
        xe8 = (16.0 * xe[T16:]).astype(F8)
        xeT8_t = xe8.T.reshape(N_DC // 2, 2, 128, T8G1).transpose(2, 0, 1, 3)

        w1_t = (W1[e].astype(np.float16)
                .reshape(N_DC, 128, N_FBLK, F_BLK).transpose(2, 1, 0, 3))
        w18_f = (64.0 * W1[e]).astype(F8)
        w18_t = (w18_f.reshape(N_DC // 2, 2, 128, N_FBLK, F_BLK)
                 .transpose(3, 2, 0, 1, 4))

        w2s = 1024.0 * W2[e]
        w2_t = (w2s.astype(np.float16)
                .reshape(N_FBLK, N_FC, 128, D).transpose(0, 2, 1, 3))
        w28_t = (w2s.astype(F8)
                 .reshape(N_FBLK, N_FC // 2, 2, 128, D).transpose(0, 3, 1, 2, 4))

        in_maps.append({
            "xeT": np.ascontiguousarray(xeT_t),
            "xeT8": np.ascontiguousarray(xeT8_t),
            "w1": np.ascontiguousarray(w1_t),
            "w18": np.ascontiguousarray(w18_t),
            "w2": np.ascontiguousarray(w2_t),
            "w28": np.ascontiguousarray(w28_t),
        })

    nc = _get_nc()
    res = run_bass_kernel_spmd(nc, in_maps, core_ids=list(range(E)))

    yf = np.zeros((T, D), dtype=np.float32)
    for e in range(E):
        ye = res.results[e]["y"].astype(np.float32).sum(axis=0)  # [CAP, D]
        m = valid_s[e]
        yf[disp_s[e][m]] = ye[m]
    return yf.reshape(B, S, D)
